# revision 69
# baseline (speedup 1.0000x reference)
"""Trainium2 Bass kernel for the HPM gaussian-ray read problem (sparse).

out[b,c] = sum_n exp(-r2[n,b]/(2*sigma^2)) * exp(-max(t[n,b],0)/tau) * mem[n,c]

over the flattened 128^3 grid (N = 2,097,152), B=32 rays, C=16 channels.

Structure: for a fixed grid column (gx,gy) the log-weight W is, in z,
min(W0, W1) of two quadratics (W1 = W0 - t/tau; t>0 <=> W1<W0).  The
Gaussian factor (sigma=0.5 voxels) confines significant weight to a
narrow z-window per (column, ray), so on host (f64, cheap: O(N^(2/3)*B))
every (column, ray) pair is classified:

  - inactive (~98.3%): total weight mass < 1e-4  -> dropped entirely
  - single-branch (~1.7%): the t=0 kink lies outside the active window,
    so ONE quadratic is exact where it matters -> one device pair-slot
  - straddle (~0.02%): kink inside the window -> exact min(W0,W1) via a
    small dedicated pair/min block on device

Device kernel per core (all cores run the same static program):
  - 4 groups x 32 tiles; tile = 8 active columns' memory [128z, 8x16c]
    (bf16) + 16 pair-slots.
  - mm1 per group: static 11-row bf16 basis [1,u,uh,ul triplet splits,
    ~24 mantissa bits] x coef [11, 512] -> psW [128 z, 512 slots] fp32.
  - ACT exp -> kern bf16; per tile: psO[:,16t:16t+16] = mem_tile^T @
    kern_slots (stationary = mem tile, 16-col moving operand).
  - straddle block: 32 pair-slots x 2 branches -> DVE pairwise min ->
    exp -> 4 straddle tiles.
  - psO evacuated by DVE, DMAed out via gpsimd (SWDGE) ring; memory
    tiles arrive as 4 x 1MB DMAs on the sync (HWDGE) ring.
Host gathers psO columns (each column = one pair's per-channel sums at
its tile row block) and scatter-adds into out[B, C].

Sharding: active columns are distributed over the 8 cores (balanced to
fit the static 128-tile capacity); each core sees only its columns'
memory. The [B,C] partials are summed on host.
"""

import numpy as np

SIGMA = 0.5
TAU = 2.0
NCORES = 8
D = 128           # grid edge / z depth
B = 32            # rays
C = 16            # channels
KROWS = 11        # split-bf16 basis rows
NCH = D * D       # 16384 (gx,gy) columns

NGRP = 4          # regular groups
TPG = 32          # tiles per group
NTILE = NGRP * TPG          # 128 regular tiles per core
TCOLS = 8                   # column entries per tile
KP = 16                     # pair slots per tile
REG_SLOTS = NTILE * KP      # 2048
GSLOTS = TPG * KP           # 512 slots per group
S_CAP = 32                  # straddle pair slots per core
NSTILE = 4                  # straddle tiles (8 entries, 8 slots each)
SPT = 8                     # straddle slots per tile
EPS = 1e-4                  # per-pair neglected-mass threshold
YTHR = 1e-8                 # per-z weight threshold for z-windows
WNEG = -30000.0             # "minus infinity" log-weight
ZS = 32                     # z rows per entry
KR2 = 8                     # basis rows [C*3, B*3, A*2]

_BASS_CACHE = {}


# ---------------------------------------------------------------- device ---

def _build_nc():
    from contextlib import ExitStack
    import concourse.bacc as bacc
    import concourse.mybir as mybir
    from concourse.tile import TileContext

    f32 = mybir.dt.float32
    bf16 = mybir.dt.bfloat16
    nc = bacc.Bacc()
    zaug_d = nc.dram_tensor("zaug", [KR2, ZS], bf16, kind="ExternalInput")
    coef_d = nc.dram_tensor("coef", [KR2, REG_SLOTS + 2 * S_CAP], bf16,
                            kind="ExternalInput")
    mem_d = nc.dram_tensor("mem", [NGRP, ZS, TPG * TCOLS * C], bf16,
                           kind="ExternalInput")
    smem_d = nc.dram_tensor("smem", [ZS, NSTILE * TCOLS * C], bf16,
                            kind="ExternalInput")
    # out layout: [g, 32*(ti%4)+slot, 128*(ti//4)+16*j+c]; straddle at
    # [NGRP, 32*t+slot, 16*j+c].
    out_d = nc.dram_tensor("out", [NGRP + 1, D, TPG * 32], f32,
                           kind="ExternalOutput")
    Exp = mybir.ActivationFunctionType.Exp

    with TileContext(nc) as tc:
        with ExitStack() as ctx:
            singles = ctx.enter_context(tc.tile_pool(name="singles", bufs=1))
            pswpool = ctx.enter_context(
                tc.tile_pool(name="psw", bufs=3, space="PSUM"))
            psopool = ctx.enter_context(
                tc.tile_pool(name="pso", bufs=2, space="PSUM"))
            psospool = ctx.enter_context(
                tc.tile_pool(name="psos", bufs=1, space="PSUM"))

            coefsb = singles.tile([KR2, REG_SLOTS + 2 * S_CAP], bf16)
            nc.gpsimd.dma_start(out=coefsb[:], in_=coef_d[:, :])
            zaug = singles.tile([KR2, ZS], bf16)
            nc.gpsimd.dma_start(out=zaug[:], in_=zaug_d[:, :])
            smem = singles.tile([ZS, NSTILE * TCOLS * C], bf16)
            nc.scalar.dma_start(out=smem[:], in_=smem_d[:, :])
            # memory tiles spread over three DMA rings (sync/scalar/gpsimd)
            memsb = [singles.tile([ZS, TPG * TCOLS * C], bf16,
                                  name=f"memsb{g}") for g in range(NGRP)]
            nc.sync.dma_start(out=memsb[0][:], in_=mem_d[0])
            nc.gpsimd.dma_start(out=memsb[1][:], in_=mem_d[1])
            nc.scalar.dma_start(out=memsb[2][:], in_=mem_d[2])
            nc.scalar.dma_start(out=memsb[3][:], in_=mem_d[3])

            kerns = [singles.tile([ZS, GSLOTS], bf16, name=f"kern{g}")
                     for g in range(NGRP)]
            kern_s = singles.tile([ZS, S_CAP], bf16)
            wm_s = singles.tile([ZS, S_CAP], f32)
            outsb = [singles.tile([D, TPG * 32], f32, name=f"outsb{g}")
                     for g in range(NGRP)]
            outsb_s = singles.tile([D, TCOLS * C], f32)

            psW = [None] * NGRP

            def mm1(g):
                psW[g] = pswpool.tile([ZS, GSLOTS], f32, name=f"psW{g}",
                                      tag="psW")
                nc.tensor.matmul(psW[g][:], zaug[:],
                                 coefsb[:, g * GSLOTS:(g + 1) * GSLOTS],
                                 start=True, stop=True)

            def mm2(g):
                # stationary = kern slots (16 cols, cheap LDWEIGHTS),
                # moving = mem tile; 4-strip col tiling so LDWEIGHTS and
                # matmuls of adjacent tiles overlap in the PE array.
                psO = psopool.tile([D, TPG * 32], f32, name=f"psO{g}",
                                   tag="psO")
                engs = ((nc.sync, nc.sync), (nc.gpsimd, nc.gpsimd),
                        (nc.scalar, nc.sync), (nc.scalar, nc.gpsimd))[g]
                for h in range(2):
                    for th in range(TPG // 2):
                        t = TPG // 2 * h + th
                        s = t % 4
                        nc.tensor.matmul(
                            psO[32 * s:32 * s + KP,
                                128 * (t // 4):128 * (t // 4 + 1)],
                            kerns[g][:, KP * t:KP * (t + 1)],
                            memsb[g][:, 128 * t:128 * (t + 1)],
                            start=True, stop=True,
                            tile_position=(0, 32 * s))
                    half = slice(512 * h, 512 * (h + 1))
                    if h == 0:
                        nc.vector.tensor_copy(outsb[g][:, half],
                                              psO[:, half])
                    else:
                        nc.scalar.copy(out=outsb[g][:, half],
                                       in_=psO[:, half])
                    engs[h].dma_start(out=out_d[g][:, half],
                                      in_=outsb[g][:, half])

            # warm-up matmuls on garbage data while inputs stream in:
            # ~4 us of sustained PE activity releases the HAM clock gate
            # (1.2 -> 2.4 GHz) before the real matmuls start.
            for w in range(8):
                pwarm = pswpool.tile([D, GSLOTS], f32, name=f"warm{w}",
                                     tag="psW")
                nc.tensor.matmul(pwarm[:], kerns[0][0:32, 0:128],
                                 kerns[0][0:32, 0:GSLOTS],
                                 start=True, stop=True)

            mm1(0)
            mm1(1)
            # straddle block early: 32 (W0, W1) pairs -> min -> exp -> mm2
            psW_s = pswpool.tile([ZS, 2 * S_CAP], f32, tag="psW")
            nc.tensor.matmul(psW_s[:], zaug[:],
                             coefsb[:, REG_SLOTS:REG_SLOTS + 2 * S_CAP],
                             start=True, stop=True)
            pw = psW_s[:].rearrange("p (s two) -> p s two", two=2)
            nc.vector.tensor_reduce(wm_s[:], pw, axis=mybir.AxisListType.X,
                                    op=mybir.AluOpType.min)
            nc.scalar.activation(kerns[0][:], psW[0][:], Exp)
            nc.scalar.activation(kern_s[:], wm_s[:], Exp)
            psO_s = psospool.tile([D, TCOLS * C], f32)
            for t in range(NSTILE):
                nc.tensor.matmul(
                    psO_s[32 * t:32 * t + SPT, :],
                    kern_s[:, SPT * t:SPT * (t + 1)],
                    smem[:, 128 * t:128 * (t + 1)],
                    start=True, stop=True,
                    tile_position=(0, 32 * t))
            nc.vector.tensor_copy(outsb_s[:], psO_s[:])
            nc.gpsimd.dma_start(out=out_d[NGRP, :, 0:TCOLS * C],
                                in_=outsb_s[:])

            mm2(0)
            mm1(2)
            nc.scalar.activation(kerns[1][:], psW[1][:], Exp)
            mm2(1)
            mm1(3)
            nc.scalar.activation(kerns[2][:], psW[2][:], Exp)
            mm2(2)
            nc.scalar.activation(kerns[3][:], psW[3][:], Exp)
            mm2(3)

    nc.compile()
    return nc


def _get_nc():
    if "nc" not in _BASS_CACHE:
        _BASS_CACHE["nc"] = _build_nc()
    return _BASS_CACHE["nc"]


# ------------------------------------------------------------------ host ---

def _bf16(x):
    import ml_dtypes
    return np.asarray(x).astype(ml_dtypes.bfloat16)


def _split3(x):
    """f64 -> three bf16 parts summing to ~24 mantissa bits of x."""
    x0 = _bf16(x).astype(np.float64)
    x1 = _bf16(x - x0).astype(np.float64)
    x2 = _bf16(x - x0 - x1).astype(np.float64)
    return x0, x1, x2


def _pack_cols(Aq, Bq, Cq, zb):
    """f64 quadratic in u_orig = z - 64 -> [8, n] bf16 split rows,
    recentered to the entry starting at zb.
    Rows: [C0,C1,C2, B0,B1,B2, A0,A1]."""
    Aq = np.asarray(Aq, np.float64)
    Bq = np.asarray(Bq, np.float64)
    Cq = np.asarray(Cq, np.float64)
    zb = np.asarray(zb, np.float64)
    cs = zb + 16 - 64
    Bt = Bq + 2 * Aq * cs
    Ct = Cq + Bq * cs + Aq * cs * cs
    C_0, C_1, C_2 = _split3(Ct)
    B_0, B_1, B_2 = _split3(Bt)
    A_0, A_1, _ = _split3(Aq)
    rows = [C_0, C_1, C_2, B_0, B_1, B_2, A_0, A_1]
    return np.stack([_bf16(r) for r in rows])


def _zaug_rows():
    """Basis [8, 32]: rows [1,1,1, u,u,u, u2,u2], u = zz - 16."""
    u = np.arange(ZS, dtype=np.float64) - 16.0
    one = np.ones(ZS)
    rows = [one, one, one, u, u, u, u * u, u * u]
    return _bf16(np.stack(rows))


def _analyze(ray_origin, ray_dir):
    """Quadratic coeffs (f64) + per-(col, ray) branch assignment.

    assign: 0=inactive, 1=W0 branch, 2=W1 branch, 3=straddle."""
    o = ray_origin.astype(np.float64)
    d = ray_dir.astype(np.float64)
    d2 = (d * d).sum(-1)
    kap = 2.0 - d2
    od = (o * d).sum(-1)
    g = np.arange(D, dtype=np.float64)
    gx = np.repeat(g, D)
    gy = np.tile(g, D)
    c1 = 1.0 / (2 * SIGMA ** 2)
    c3 = 1.0 / TAU
    alpha = gx[:, None] * d[None, :, 0] + gy[:, None] * d[None, :, 1] - od[None, :]
    t64 = 64.0 * d[None, :, 2] + alpha                      # [NCH, B]
    e = 64.0 - o[:, 2]
    gamma = (gx[:, None] - o[None, :, 0]) ** 2 + (gy[:, None] - o[None, :, 1]) ** 2
    A0 = np.broadcast_to((-c1 + c1 * kap * d[:, 2] ** 2)[None, :], t64.shape)
    B0 = -2 * c1 * e[None, :] + 2 * c1 * kap[None, :] * d[None, :, 2] * t64
    C0 = -c1 * (gamma + e[None, :] ** 2) + c1 * kap[None, :] * t64 ** 2
    B1 = B0 - c3 * d[None, :, 2]
    C1 = C0 - c3 * t64

    u = np.arange(D, dtype=np.float64) - 64.0
    assign = np.zeros((NCH, B), np.int8)
    lo = np.full((NCH, B), D - 1, np.int32)
    hi = np.zeros((NCH, B), np.int32)
    CH = 2048
    for s in range(0, NCH, CH):
        sl = slice(s, s + CH)
        W0 = (A0[sl, :, None] * u[None, None, :] ** 2
              + B0[sl, :, None] * u[None, None, :] + C0[sl, :, None])
        W1 = (A0[sl, :, None] * u[None, None, :] ** 2
              + B1[sl, :, None] * u[None, None, :] + C1[sl, :, None])
        y0 = np.exp(np.minimum(W0, 50.0))
        y1 = np.exp(np.minimum(W1, 50.0))
        yt = np.minimum(y0, y1)
        Ec = yt.sum(-1)
        E0 = (y0 - yt).sum(-1)
        E1 = (y1 - yt).sum(-1)
        a = np.full(Ec.shape, 3, np.int8)
        a[E1 <= EPS] = 2
        a[E0 <= EPS] = 1
        a[Ec <= EPS] = 0
        assign[sl] = a
        m = yt >= YTHR
        any_ = m.any(-1)
        lo[sl] = np.where(any_, m.argmax(-1), D - 1)
        hi[sl] = np.where(any_, D - 1 - m[:, :, ::-1].argmax(-1), 0)
    return assign, lo, hi, (A0, B0, C0, B1, C1)


def _col_entries(col, rays, lo, hi):
    """(col, zb, rays-subset) entries with disjoint z coverage."""
    clo = int(lo[col, rays].min())
    chi = int(hi[col, rays].max())
    if chi - clo + 1 <= ZS:
        zb = min(clo, D - ZS)
        return [(col, zb, list(rays))]
    # wide column: ZS-aligned blocks (disjoint)
    out = []
    for k in range(clo // ZS, chi // ZS + 1):
        rs = [r for r in rays
              if lo[col, r] < ZS * (k + 1) and hi[col, r] >= ZS * k]
        if rs:
            out.append((col, ZS * k, rs))
    return out


def _pack_tiles(entries, max_cols, max_pairs):
    """First-fit entries into tiles; entries may split across tiles."""
    tiles = []   # [entry list of (col, zb), pair list of (j, ray)]
    for col, zb, rays in entries:
        pos = 0
        while True:
            take = rays[pos:]
            placed = False
            for tl in tiles:
                room = max_pairs - len(tl[1])
                if len(tl[0]) < max_cols and room > 0:
                    j = len(tl[0])
                    tl[0].append((col, zb))
                    for r in take[:room]:
                        tl[1].append((j, int(r)))
                    pos += min(len(take), room)
                    placed = True
                    break
            if not placed:
                tiles.append([[], []])
                continue
            if pos >= len(rays):
                break
    return tiles


def _plan(assign, lo, hi):
    """Column -> core assignment and per-core entry/tile packing."""
    act = assign != 0
    straddle = assign == 3
    reg_pairs_per_col = ((assign == 1) | (assign == 2)).sum(1)
    col_active = act.any(1)
    cols = np.nonzero(col_active)[0]
    order = cols[np.argsort(-reg_pairs_per_col[cols], kind="stable")]
    loads = np.zeros(NCORES, np.int64)
    colcnt = np.zeros(NCORES, np.int64)
    core_cols = [[] for _ in range(NCORES)]
    for col in order:
        k = int(np.lexsort((colcnt, loads))[0])
        core_cols[k].append(col)
        loads[k] += reg_pairs_per_col[col]
        colcnt[k] += 1

    plans = []
    for k in range(NCORES):
        entries = []
        for col in sorted(core_cols[k]):
            rays = np.nonzero((assign[col] == 1) | (assign[col] == 2))[0]
            if len(rays):
                entries += _col_entries(col, rays, lo, hi)
        tiles = _pack_tiles(entries, TCOLS, KP)
        assert len(tiles) <= NTILE, f"core {k}: {len(tiles)} tiles > {NTILE}"
        sentries = []
        for col in sorted(core_cols[k]):
            rays = np.nonzero(straddle[col])[0]
            if len(rays):
                sentries += _col_entries(col, rays, lo, hi)
        stiles = _pack_tiles(sentries, TCOLS, SPT)
        assert len(stiles) <= NSTILE, f"core {k}: straddle tiles {len(stiles)}"
        plans.append((tiles, stiles))
    return plans


def _prep_inputs(ray_origin, ray_dir, memory):
    import ml_dtypes
    assign, lo, hi, (A0, B0, C0, B1, C1) = _analyze(ray_origin, ray_dir)
    plans = _plan(assign, lo, hi)
    zaug = _zaug_rows()
    mem = np.ascontiguousarray(memory, dtype=np.float32).reshape(NCH, D, C)
    mem_bf = mem.astype(ml_dtypes.bfloat16)
    const_col = _pack_cols(np.zeros(1), np.zeros(1), np.full(1, WNEG),
                           np.zeros(1))[:, 0]

    in_maps = []
    extracts = []
    for k in range(NCORES):
        tiles, stiles = plans[k]
        memg = np.zeros((NGRP, ZS, TPG * TCOLS * C), ml_dtypes.bfloat16)
        coef = np.tile(const_col[:, None],
                       (1, REG_SLOTS + 2 * S_CAP)).astype(ml_dtypes.bfloat16)
        smemg = np.zeros((ZS, NSTILE * TCOLS * C), ml_dtypes.bfloat16)
        ext_row, ext_col, ext_ray = [], [], []   # psO row, col base, ray
        for t, (tents, tpairs) in enumerate(tiles):
            g, ti = divmod(t, TPG)
            for j, (col, zb) in enumerate(tents):
                memg[g, :, (ti * TCOLS + j) * C:(ti * TCOLS + j + 1) * C] = \
                    mem_bf[col, zb:zb + ZS]
            if tpairs:
                js = np.array([p[0] for p in tpairs])
                rs = np.array([p[1] for p in tpairs])
                colids = np.array([tents[j][0] for j in js])
                zbs = np.array([tents[j][1] for j in js])
                brs = assign[colids, rs]
                Bq = np.where(brs == 1, B0[colids, rs], B1[colids, rs])
                Cq = np.where(brs == 1, C0[colids, rs], C1[colids, rs])
                cc = _pack_cols(A0[colids, rs], Bq, Cq, zbs)
                coef[:, t * KP:t * KP + len(tpairs)] = cc
                si = np.arange(len(tpairs))
                ext_row += list(128 * g + 32 * (ti % 4) + si)
                ext_col += list(128 * (ti // 4) + 16 * js)
                ext_ray += list(rs)
        s_row, s_col, s_ray = [], [], []
        for t, (tents, tpairs) in enumerate(stiles):
            for j, (col, zb) in enumerate(tents):
                smemg[:, (t * TCOLS + j) * C:(t * TCOLS + j + 1) * C] = \
                    mem_bf[col, zb:zb + ZS]
            for si, (j, r) in enumerate(tpairs):
                slot = t * SPT + si
                col, zb = tents[j]
                c0 = _pack_cols(A0[col:col + 1, r], B0[col:col + 1, r],
                                C0[col:col + 1, r], np.full(1, zb))[:, 0]
                c1 = _pack_cols(A0[col:col + 1, r], B1[col:col + 1, r],
                                C1[col:col + 1, r], np.full(1, zb))[:, 0]
                coef[:, REG_SLOTS + 2 * slot] = c0
                coef[:, REG_SLOTS + 2 * slot + 1] = c1
                s_row.append(128 * NGRP + 32 * t + si)
                s_col.append(16 * j)
                s_ray.append(r)
        in_maps.append({"zaug": zaug,
                        "coef": np.ascontiguousarray(coef),
                        "mem": memg,
                        "smem": smemg})
        extracts.append((np.array(ext_row, np.int64),
                         np.array(ext_col, np.int64),
                         np.array(ext_ray, np.int64),
                         np.array(s_row, np.int64),
                         np.array(s_col, np.int64),
                         np.array(s_ray, np.int64)))
    return in_maps, extracts


def _extract(results, extracts):
    out = np.zeros((B, C), np.float64)
    r16 = np.arange(16)
    for res, (row, col, ray, srow, scol, sray) in zip(results, extracts):
        ps = res["out"].astype(np.float64).reshape((NGRP + 1) * D, TPG * 32)
        if len(row):
            vals = ps[row[:, None], col[:, None] + r16[None, :]]
            np.add.at(out, ray, vals)
        if len(srow):
            vals = ps[srow[:, None], scol[:, None] + r16[None, :]]
            np.add.at(out, sray, vals)
    return out.astype(np.float32)


def emulate(ray_origin, ray_dir, memory):
    """Numpy emulation of the device program (packing/index validation)."""
    in_maps, extracts = _prep_inputs(ray_origin, ray_dir, memory)
    results = []
    for im in in_maps:
        zaug = im["zaug"].astype(np.float64)
        coef = im["coef"].astype(np.float64)
        psW = zaug.T @ coef                     # [64, 2112]
        kern = np.exp(psW[:, :REG_SLOTS])
        pws = psW[:, REG_SLOTS:].reshape(ZS, S_CAP, 2)
        kern_s = np.exp(pws.min(-1))
        kern = _bf16(kern).astype(np.float64)
        kern_s = _bf16(kern_s).astype(np.float64)
        out = np.zeros((NGRP + 1, D, TPG * 32), np.float64)
        memg = im["mem"].astype(np.float64)
        for t in range(NTILE):
            g, ti = divmod(t, TPG)
            mt = memg[g][:, 128 * ti:128 * (ti + 1)]
            blk = kern[:, KP * t:KP * (t + 1)].T @ mt       # [16, 128]
            r0 = 32 * (ti % 4)
            c0 = 128 * (ti // 4)
            out[g, r0:r0 + KP, c0:c0 + 128] = blk
        smem = im["smem"].astype(np.float64)
        for t in range(NSTILE):
            mt = smem[:, 128 * t:128 * (t + 1)]
            blk = kern_s[:, SPT * t:SPT * (t + 1)].T @ mt   # [8, 128]
            out[NGRP, 32 * t:32 * t + SPT, 0:128] = blk
        results.append({"out": out.astype(np.float32)})
    return _extract(results, extracts)


def run_kernel(ray_origin, ray_dir, memory, trace=False, **run_kwargs):
    """Run on 8 NeuronCores; returns ([B,C] output, BassKernelResults)."""
    from concourse.bass_utils import run_bass_kernel_spmd
    nc = _get_nc()
    in_maps, extracts = _prep_inputs(np.asarray(ray_origin),
                                     np.asarray(ray_dir),
                                     np.asarray(memory))
    br = run_bass_kernel_spmd(nc, in_maps, core_ids=list(range(NCORES)),
                              trace=trace, **run_kwargs)
    return _extract(br.results, extracts), br


def kernel(ray_origin, ray_dir, memory):
    out, _ = run_kernel(np.asarray(ray_origin), np.asarray(ray_dir),
                        np.asarray(memory))
    return out


# revision 70
# speedup vs baseline: 1.2397x; 1.2397x over previous
"""Trainium2 Bass kernel for the HPM gaussian-ray read problem (sparse).

out[b,c] = sum_n exp(-r2[n,b]/(2*sigma^2)) * exp(-max(t[n,b],0)/tau) * mem[n,c]

over the flattened 128^3 grid (N = 2,097,152), B=32 rays, C=16 channels.

Structure: for a fixed grid column (gx,gy) the log-weight W is, in z,
min(W0, W1) of two quadratics (W1 = W0 - t/tau; t>0 <=> W1<W0).  The
Gaussian factor (sigma=0.5 voxels) confines significant weight to a
narrow z-window per (column, ray), so on host (f64, cheap: O(N^(2/3)*B))
every (column, ray) pair is classified:

  - inactive (~98.3%): total weight mass < 1e-4  -> dropped entirely
  - single-branch (~1.7%): the t=0 kink lies outside the active window,
    so ONE quadratic is exact where it matters -> one device pair-slot
  - straddle (~0.02%): kink inside the window -> exact min(W0,W1) via a
    small dedicated pair/min block on device

Device kernel per core (all cores run the same static program):
  - 4 groups x 32 tiles; tile = 8 active columns' memory [128z, 8x16c]
    (bf16) + 16 pair-slots.
  - mm1 per group: static 11-row bf16 basis [1,u,uh,ul triplet splits,
    ~24 mantissa bits] x coef [11, 512] -> psW [128 z, 512 slots] fp32.
  - ACT exp -> kern bf16; per tile: psO[:,16t:16t+16] = mem_tile^T @
    kern_slots (stationary = mem tile, 16-col moving operand).
  - straddle block: 32 pair-slots x 2 branches -> DVE pairwise min ->
    exp -> 4 straddle tiles.
  - psO evacuated by DVE, DMAed out via gpsimd (SWDGE) ring; memory
    tiles arrive as 4 x 1MB DMAs on the sync (HWDGE) ring.
Host gathers psO columns (each column = one pair's per-channel sums at
its tile row block) and scatter-adds into out[B, C].

Sharding: active columns are distributed over the 8 cores (balanced to
fit the static 128-tile capacity); each core sees only its columns'
memory. The [B,C] partials are summed on host.
"""

import numpy as np

SIGMA = 0.5
TAU = 2.0
NCORES = 8
D = 128           # grid edge / z depth
B = 32            # rays
C = 16            # channels
KROWS = 11        # split-bf16 basis rows
NCH = D * D       # 16384 (gx,gy) columns

NGRP = 4          # regular groups
TPG = 32          # tiles per group
NTILE = NGRP * TPG          # 128 regular tiles per core
TCOLS = 8                   # column entries per tile
KP = 16                     # pair slots per tile
REG_SLOTS = NTILE * KP      # 2048
GSLOTS = TPG * KP           # 512 slots per group
S_CAP = 32                  # straddle pair slots per core
NSTILE = 4                  # straddle tiles (8 entries, 8 slots each)
SPT = 8                     # straddle slots per tile
EPS = 1e-4                  # per-pair neglected-mass threshold
YTHR = 1e-8                 # per-z weight threshold for z-windows
WNEG = -30000.0             # "minus infinity" log-weight
ZS = 32                     # z rows per entry
KR2 = 8                     # basis rows [C*3, B*3, A*2]

_BASS_CACHE = {}


# ---------------------------------------------------------------- device ---

def _build_nc():
    from contextlib import ExitStack
    import concourse.bacc as bacc
    import concourse.mybir as mybir
    from concourse.tile import TileContext

    f32 = mybir.dt.float32
    bf16 = mybir.dt.bfloat16
    nc = bacc.Bacc()
    zaug_d = nc.dram_tensor("zaug", [KR2, ZS], bf16, kind="ExternalInput")
    coef_d = nc.dram_tensor("coef", [KR2, REG_SLOTS + 2 * S_CAP], bf16,
                            kind="ExternalInput")
    mem_d = nc.dram_tensor("mem", [NGRP, ZS, TPG * TCOLS * C], bf16,
                           kind="ExternalInput")
    smem_d = nc.dram_tensor("smem", [ZS, NSTILE * TCOLS * C], bf16,
                            kind="ExternalInput")
    # out layout: [g, 32*(ti%4)+slot, 128*(ti//4)+16*j+c]; straddle at
    # [NGRP, 32*t+slot, 16*j+c].
    out_d = nc.dram_tensor("out", [NGRP + 1, D, TPG * 32], f32,
                           kind="ExternalOutput")
    Exp = mybir.ActivationFunctionType.Exp

    with TileContext(nc) as tc:
        with ExitStack() as ctx:
            singles = ctx.enter_context(tc.tile_pool(name="singles", bufs=1))
            pswpool = ctx.enter_context(
                tc.tile_pool(name="psw", bufs=3, space="PSUM"))
            psopool = ctx.enter_context(
                tc.tile_pool(name="pso", bufs=2, space="PSUM"))
            psospool = ctx.enter_context(
                tc.tile_pool(name="psos", bufs=1, space="PSUM"))

            coefsb = singles.tile([KR2, REG_SLOTS + 2 * S_CAP], bf16)
            nc.gpsimd.dma_start(out=coefsb[:], in_=coef_d[:, :])
            zaug = singles.tile([KR2, ZS], bf16)
            nc.gpsimd.dma_start(out=zaug[:], in_=zaug_d[:, :])
            smem = singles.tile([ZS, NSTILE * TCOLS * C], bf16)
            nc.scalar.dma_start(out=smem[:], in_=smem_d[:, :])
            # memory tiles spread over three DMA rings (sync/scalar/gpsimd)
            memsb = [singles.tile([ZS, TPG * TCOLS * C], bf16,
                                  name=f"memsb{g}") for g in range(NGRP)]
            nc.sync.dma_start(out=memsb[0][:], in_=mem_d[0])
            nc.gpsimd.dma_start(out=memsb[1][:], in_=mem_d[1])
            nc.scalar.dma_start(out=memsb[2][:], in_=mem_d[2])
            nc.scalar.dma_start(out=memsb[3][:], in_=mem_d[3])

            kerns = [singles.tile([ZS, GSLOTS], bf16, name=f"kern{g}")
                     for g in range(NGRP)]
            kern_s = singles.tile([ZS, S_CAP], bf16)
            wm_s = singles.tile([ZS, S_CAP], f32)
            outsb = [singles.tile([D, TPG * 32], f32, name=f"outsb{g}")
                     for g in range(NGRP)]
            outsb_s = singles.tile([D, TCOLS * C], f32)

            psW = [None] * NGRP

            def mm1(g):
                psW[g] = pswpool.tile([ZS, GSLOTS], f32, name=f"psW{g}",
                                      tag="psW")
                nc.tensor.matmul(psW[g][:], zaug[:],
                                 coefsb[:, g * GSLOTS:(g + 1) * GSLOTS],
                                 start=True, stop=True)

            def mm2(g):
                # stationary = kern slots (16 cols, cheap LDWEIGHTS),
                # moving = mem tile; 4-strip col tiling so LDWEIGHTS and
                # matmuls of adjacent tiles overlap in the PE array.
                psO = psopool.tile([D, TPG * 32], f32, name=f"psO{g}",
                                   tag="psO")
                engs = ((nc.sync, nc.sync), (nc.gpsimd, nc.gpsimd),
                        (nc.scalar, nc.sync), (nc.scalar, nc.gpsimd))[g]
                for h in range(2):
                    for th in range(TPG // 2):
                        t = TPG // 2 * h + th
                        s = t % 4
                        nc.tensor.matmul(
                            psO[32 * s:32 * s + KP,
                                128 * (t // 4):128 * (t // 4 + 1)],
                            kerns[g][:, KP * t:KP * (t + 1)],
                            memsb[g][:, 128 * t:128 * (t + 1)],
                            start=True, stop=True,
                            tile_position=(0, 32 * s))
                    half = slice(512 * h, 512 * (h + 1))
                    if h == 0:
                        nc.vector.tensor_copy(outsb[g][:, half],
                                              psO[:, half])
                    else:
                        nc.scalar.copy(out=outsb[g][:, half],
                                       in_=psO[:, half])
                    engs[h].dma_start(out=out_d[g][:, half],
                                      in_=outsb[g][:, half])

            mm1(0)
            mm1(1)
            # straddle block early: 32 (W0, W1) pairs -> min -> exp -> mm2
            psW_s = pswpool.tile([ZS, 2 * S_CAP], f32, tag="psW")
            nc.tensor.matmul(psW_s[:], zaug[:],
                             coefsb[:, REG_SLOTS:REG_SLOTS + 2 * S_CAP],
                             start=True, stop=True)
            pw = psW_s[:].rearrange("p (s two) -> p s two", two=2)
            nc.vector.tensor_reduce(wm_s[:], pw, axis=mybir.AxisListType.X,
                                    op=mybir.AluOpType.min)
            nc.scalar.activation(kerns[0][:], psW[0][:], Exp)
            nc.scalar.activation(kern_s[:], wm_s[:], Exp)
            psO_s = psospool.tile([D, TCOLS * C], f32)
            for t in range(NSTILE):
                nc.tensor.matmul(
                    psO_s[32 * t:32 * t + SPT, :],
                    kern_s[:, SPT * t:SPT * (t + 1)],
                    smem[:, 128 * t:128 * (t + 1)],
                    start=True, stop=True,
                    tile_position=(0, 32 * t))
            nc.vector.tensor_copy(outsb_s[:], psO_s[:])
            nc.gpsimd.dma_start(out=out_d[NGRP, :, 0:TCOLS * C],
                                in_=outsb_s[:])

            mm2(0)
            mm1(2)
            nc.scalar.activation(kerns[1][:], psW[1][:], Exp)
            mm2(1)
            mm1(3)
            nc.scalar.activation(kerns[2][:], psW[2][:], Exp)
            mm2(2)
            nc.scalar.activation(kerns[3][:], psW[3][:], Exp)
            mm2(3)

    nc.compile()
    return nc


def _get_nc():
    if "nc" not in _BASS_CACHE:
        _BASS_CACHE["nc"] = _build_nc()
    return _BASS_CACHE["nc"]


# ------------------------------------------------------------------ host ---

def _bf16(x):
    import ml_dtypes
    return np.asarray(x).astype(ml_dtypes.bfloat16)


def _split3(x):
    """f64 -> three bf16 parts summing to ~24 mantissa bits of x."""
    x0 = _bf16(x).astype(np.float64)
    x1 = _bf16(x - x0).astype(np.float64)
    x2 = _bf16(x - x0 - x1).astype(np.float64)
    return x0, x1, x2


def _pack_cols(Aq, Bq, Cq, zb):
    """f64 quadratic in u_orig = z - 64 -> [8, n] bf16 split rows,
    recentered to the entry starting at zb.
    Rows: [C0,C1,C2, B0,B1,B2, A0,A1]."""
    Aq = np.asarray(Aq, np.float64)
    Bq = np.asarray(Bq, np.float64)
    Cq = np.asarray(Cq, np.float64)
    zb = np.asarray(zb, np.float64)
    cs = zb + 16 - 64
    Bt = Bq + 2 * Aq * cs
    Ct = Cq + Bq * cs + Aq * cs * cs
    C_0, C_1, C_2 = _split3(Ct)
    B_0, B_1, B_2 = _split3(Bt)
    A_0, A_1, _ = _split3(Aq)
    rows = [C_0, C_1, C_2, B_0, B_1, B_2, A_0, A_1]
    return np.stack([_bf16(r) for r in rows])


def _zaug_rows():
    """Basis [8, 32]: rows [1,1,1, u,u,u, u2,u2], u = zz - 16."""
    u = np.arange(ZS, dtype=np.float64) - 16.0
    one = np.ones(ZS)
    rows = [one, one, one, u, u, u, u * u, u * u]
    return _bf16(np.stack(rows))


def _analyze(ray_origin, ray_dir):
    """Quadratic coeffs (f64) + per-(col, ray) branch assignment.

    assign: 0=inactive, 1=W0 branch, 2=W1 branch, 3=straddle."""
    o = ray_origin.astype(np.float64)
    d = ray_dir.astype(np.float64)
    d2 = (d * d).sum(-1)
    kap = 2.0 - d2
    od = (o * d).sum(-1)
    g = np.arange(D, dtype=np.float64)
    gx = np.repeat(g, D)
    gy = np.tile(g, D)
    c1 = 1.0 / (2 * SIGMA ** 2)
    c3 = 1.0 / TAU
    alpha = gx[:, None] * d[None, :, 0] + gy[:, None] * d[None, :, 1] - od[None, :]
    t64 = 64.0 * d[None, :, 2] + alpha                      # [NCH, B]
    e = 64.0 - o[:, 2]
    gamma = (gx[:, None] - o[None, :, 0]) ** 2 + (gy[:, None] - o[None, :, 1]) ** 2
    A0 = np.broadcast_to((-c1 + c1 * kap * d[:, 2] ** 2)[None, :], t64.shape)
    B0 = -2 * c1 * e[None, :] + 2 * c1 * kap[None, :] * d[None, :, 2] * t64
    C0 = -c1 * (gamma + e[None, :] ** 2) + c1 * kap[None, :] * t64 ** 2
    B1 = B0 - c3 * d[None, :, 2]
    C1 = C0 - c3 * t64

    u = np.arange(D, dtype=np.float64) - 64.0
    assign = np.zeros((NCH, B), np.int8)
    lo = np.full((NCH, B), D - 1, np.int32)
    hi = np.zeros((NCH, B), np.int32)
    CH = 2048
    for s in range(0, NCH, CH):
        sl = slice(s, s + CH)
        W0 = (A0[sl, :, None] * u[None, None, :] ** 2
              + B0[sl, :, None] * u[None, None, :] + C0[sl, :, None])
        W1 = (A0[sl, :, None] * u[None, None, :] ** 2
              + B1[sl, :, None] * u[None, None, :] + C1[sl, :, None])
        y0 = np.exp(np.minimum(W0, 50.0))
        y1 = np.exp(np.minimum(W1, 50.0))
        yt = np.minimum(y0, y1)
        Ec = yt.sum(-1)
        E0 = (y0 - yt).sum(-1)
        E1 = (y1 - yt).sum(-1)
        a = np.full(Ec.shape, 3, np.int8)
        a[E1 <= EPS] = 2
        a[E0 <= EPS] = 1
        a[Ec <= EPS] = 0
        assign[sl] = a
        m = yt >= YTHR
        any_ = m.any(-1)
        lo[sl] = np.where(any_, m.argmax(-1), D - 1)
        hi[sl] = np.where(any_, D - 1 - m[:, :, ::-1].argmax(-1), 0)
    return assign, lo, hi, (A0, B0, C0, B1, C1)


def _col_entries(col, rays, lo, hi):
    """(col, zb, rays-subset) entries with disjoint z coverage."""
    clo = int(lo[col, rays].min())
    chi = int(hi[col, rays].max())
    if chi - clo + 1 <= ZS:
        zb = min(clo, D - ZS)
        return [(col, zb, list(rays))]
    # wide column: ZS-aligned blocks (disjoint)
    out = []
    for k in range(clo // ZS, chi // ZS + 1):
        rs = [r for r in rays
              if lo[col, r] < ZS * (k + 1) and hi[col, r] >= ZS * k]
        if rs:
            out.append((col, ZS * k, rs))
    return out


def _pack_tiles(entries, max_cols, max_pairs):
    """First-fit entries into tiles; entries may split across tiles."""
    tiles = []   # [entry list of (col, zb), pair list of (j, ray)]
    for col, zb, rays in entries:
        pos = 0
        while True:
            take = rays[pos:]
            placed = False
            for tl in tiles:
                room = max_pairs - len(tl[1])
                if len(tl[0]) < max_cols and room > 0:
                    j = len(tl[0])
                    tl[0].append((col, zb))
                    for r in take[:room]:
                        tl[1].append((j, int(r)))
                    pos += min(len(take), room)
                    placed = True
                    break
            if not placed:
                tiles.append([[], []])
                continue
            if pos >= len(rays):
                break
    return tiles


def _plan(assign, lo, hi):
    """Column -> core assignment and per-core entry/tile packing."""
    act = assign != 0
    straddle = assign == 3
    reg_pairs_per_col = ((assign == 1) | (assign == 2)).sum(1)
    col_active = act.any(1)
    cols = np.nonzero(col_active)[0]
    order = cols[np.argsort(-reg_pairs_per_col[cols], kind="stable")]
    loads = np.zeros(NCORES, np.int64)
    colcnt = np.zeros(NCORES, np.int64)
    core_cols = [[] for _ in range(NCORES)]
    for col in order:
        k = int(np.lexsort((colcnt, loads))[0])
        core_cols[k].append(col)
        loads[k] += reg_pairs_per_col[col]
        colcnt[k] += 1

    plans = []
    for k in range(NCORES):
        entries = []
        for col in sorted(core_cols[k]):
            rays = np.nonzero((assign[col] == 1) | (assign[col] == 2))[0]
            if len(rays):
                entries += _col_entries(col, rays, lo, hi)
        tiles = _pack_tiles(entries, TCOLS, KP)
        assert len(tiles) <= NTILE, f"core {k}: {len(tiles)} tiles > {NTILE}"
        sentries = []
        for col in sorted(core_cols[k]):
            rays = np.nonzero(straddle[col])[0]
            if len(rays):
                sentries += _col_entries(col, rays, lo, hi)
        stiles = _pack_tiles(sentries, TCOLS, SPT)
        assert len(stiles) <= NSTILE, f"core {k}: straddle tiles {len(stiles)}"
        plans.append((tiles, stiles))
    return plans


def _prep_inputs(ray_origin, ray_dir, memory):
    import ml_dtypes
    assign, lo, hi, (A0, B0, C0, B1, C1) = _analyze(ray_origin, ray_dir)
    plans = _plan(assign, lo, hi)
    zaug = _zaug_rows()
    mem = np.ascontiguousarray(memory, dtype=np.float32).reshape(NCH, D, C)
    mem_bf = mem.astype(ml_dtypes.bfloat16)
    const_col = _pack_cols(np.zeros(1), np.zeros(1), np.full(1, WNEG),
                           np.zeros(1))[:, 0]

    in_maps = []
    extracts = []
    for k in range(NCORES):
        tiles, stiles = plans[k]
        memg = np.zeros((NGRP, ZS, TPG * TCOLS * C), ml_dtypes.bfloat16)
        coef = np.tile(const_col[:, None],
                       (1, REG_SLOTS + 2 * S_CAP)).astype(ml_dtypes.bfloat16)
        smemg = np.zeros((ZS, NSTILE * TCOLS * C), ml_dtypes.bfloat16)
        ext_row, ext_col, ext_ray = [], [], []   # psO row, col base, ray
        for t, (tents, tpairs) in enumerate(tiles):
            g, ti = divmod(t, TPG)
            for j, (col, zb) in enumerate(tents):
                memg[g, :, (ti * TCOLS + j) * C:(ti * TCOLS + j + 1) * C] = \
                    mem_bf[col, zb:zb + ZS]
            if tpairs:
                js = np.array([p[0] for p in tpairs])
                rs = np.array([p[1] for p in tpairs])
                colids = np.array([tents[j][0] for j in js])
                zbs = np.array([tents[j][1] for j in js])
                brs = assign[colids, rs]
                Bq = np.where(brs == 1, B0[colids, rs], B1[colids, rs])
                Cq = np.where(brs == 1, C0[colids, rs], C1[colids, rs])
                cc = _pack_cols(A0[colids, rs], Bq, Cq, zbs)
                coef[:, t * KP:t * KP + len(tpairs)] = cc
                si = np.arange(len(tpairs))
                ext_row += list(128 * g + 32 * (ti % 4) + si)
                ext_col += list(128 * (ti // 4) + 16 * js)
                ext_ray += list(rs)
        s_row, s_col, s_ray = [], [], []
        for t, (tents, tpairs) in enumerate(stiles):
            for j, (col, zb) in enumerate(tents):
                smemg[:, (t * TCOLS + j) * C:(t * TCOLS + j + 1) * C] = \
                    mem_bf[col, zb:zb + ZS]
            for si, (j, r) in enumerate(tpairs):
                slot = t * SPT + si
                col, zb = tents[j]
                c0 = _pack_cols(A0[col:col + 1, r], B0[col:col + 1, r],
                                C0[col:col + 1, r], np.full(1, zb))[:, 0]
                c1 = _pack_cols(A0[col:col + 1, r], B1[col:col + 1, r],
                                C1[col:col + 1, r], np.full(1, zb))[:, 0]
                coef[:, REG_SLOTS + 2 * slot] = c0
                coef[:, REG_SLOTS + 2 * slot + 1] = c1
                s_row.append(128 * NGRP + 32 * t + si)
                s_col.append(16 * j)
                s_ray.append(r)
        in_maps.append({"zaug": zaug,
                        "coef": np.ascontiguousarray(coef),
                        "mem": memg,
                        "smem": smemg})
        extracts.append((np.array(ext_row, np.int64),
                         np.array(ext_col, np.int64),
                         np.array(ext_ray, np.int64),
                         np.array(s_row, np.int64),
                         np.array(s_col, np.int64),
                         np.array(s_ray, np.int64)))
    return in_maps, extracts


def _extract(results, extracts):
    out = np.zeros((B, C), np.float64)
    r16 = np.arange(16)
    for res, (row, col, ray, srow, scol, sray) in zip(results, extracts):
        ps = res["out"].astype(np.float64).reshape((NGRP + 1) * D, TPG * 32)
        if len(row):
            vals = ps[row[:, None], col[:, None] + r16[None, :]]
            np.add.at(out, ray, vals)
        if len(srow):
            vals = ps[srow[:, None], scol[:, None] + r16[None, :]]
            np.add.at(out, sray, vals)
    return out.astype(np.float32)


def emulate(ray_origin, ray_dir, memory):
    """Numpy emulation of the device program (packing/index validation)."""
    in_maps, extracts = _prep_inputs(ray_origin, ray_dir, memory)
    results = []
    for im in in_maps:
        zaug = im["zaug"].astype(np.float64)
        coef = im["coef"].astype(np.float64)
        psW = zaug.T @ coef                     # [64, 2112]
        kern = np.exp(psW[:, :REG_SLOTS])
        pws = psW[:, REG_SLOTS:].reshape(ZS, S_CAP, 2)
        kern_s = np.exp(pws.min(-1))
        kern = _bf16(kern).astype(np.float64)
        kern_s = _bf16(kern_s).astype(np.float64)
        out = np.zeros((NGRP + 1, D, TPG * 32), np.float64)
        memg = im["mem"].astype(np.float64)
        for t in range(NTILE):
            g, ti = divmod(t, TPG)
            mt = memg[g][:, 128 * ti:128 * (ti + 1)]
            blk = kern[:, KP * t:KP * (t + 1)].T @ mt       # [16, 128]
            r0 = 32 * (ti % 4)
            c0 = 128 * (ti // 4)
            out[g, r0:r0 + KP, c0:c0 + 128] = blk
        smem = im["smem"].astype(np.float64)
        for t in range(NSTILE):
            mt = smem[:, 128 * t:128 * (t + 1)]
            blk = kern_s[:, SPT * t:SPT * (t + 1)].T @ mt   # [8, 128]
            out[NGRP, 32 * t:32 * t + SPT, 0:128] = blk
        results.append({"out": out.astype(np.float32)})
    return _extract(results, extracts)


def run_kernel(ray_origin, ray_dir, memory, trace=False, **run_kwargs):
    """Run on 8 NeuronCores; returns ([B,C] output, BassKernelResults)."""
    from concourse.bass_utils import run_bass_kernel_spmd
    nc = _get_nc()
    in_maps, extracts = _prep_inputs(np.asarray(ray_origin),
                                     np.asarray(ray_dir),
                                     np.asarray(memory))
    br = run_bass_kernel_spmd(nc, in_maps, core_ids=list(range(NCORES)),
                              trace=trace, **run_kwargs)
    return _extract(br.results, extracts), br


def kernel(ray_origin, ray_dir, memory):
    out, _ = run_kernel(np.asarray(ray_origin), np.asarray(ray_dir),
                        np.asarray(memory))
    return out
